# revision 45
# baseline (speedup 1.0000x reference)
"""Balanced EMD loss kernel for Trainium2 (8 NeuronCores, data parallel).

Math (per sample, classes w = 1..10):
    score = sum(pt * w);  var = sum(pt * (w - score)^2) = Z2 - Z1^2  (S0 == 1)
    D_k = CDF_k(pe) - CDF_k(pt) = sum_{c<=k} (pe_c - pt_c)
    emd = sqrt(mean_k D_k^2);  loss = sum(emd / var) / B

Layout: class-major, host pre-transposed.  SBUF holds X[(s*10+c), j] =
x[sample(j*12+s), c] for 12 slots x 10 classes = 120 partitions; each
column j carries 12 samples.  pe is fp8 (e4m3), pt fp8 too -- the
loss is a mean over 4M samples so quantization noise averages out
(measured ~4e-3 against the f32 reference; the gate is 2e-2).

Engine split per tile (33 chunks of 128 columns, 9 square groups):
  TensorE: D = Tbd^T @ pe - Tbd^T @ pt via two accumulating matmuls with
           a constant block-diagonal lower-triangular stationary (the CDF
           transform); per chunk a tiny data-stationary moment matmul
           (z1,z2; moving = block-diag class weights) inlined with the cdf
           stream, and one ssq matmul (sum_k D^2 over the squared cdf;
           moving = block-diag ones) in a block at the tile end, by which
           time every square has finished.  Both land sample-major in PSUM.
  The D^2 square is spread over the other engines under the PSUM access
  rules (GPSIMD: no PSUM; DVE: at most one PSUM input per instruction):
  ScalarE squares 5 groups directly; DVE copies 4 groups to SBUF fp16, of
  which GpSimd squares 3 and DVE itself squares 1 (2x mode).
  Finishing per tile (sample-major [128, 396]), deferred into the next
  tile so it never blocks a psD-releasing square: GpSimd var = z2 - z1^2,
  VectorE 1/var + (emd*weight) + reduce, ScalarE emd = sqrt(0.1*ssq) and
  the moment-PSUM copies (split with VectorE).
"""

import numpy as np

P = 128          # sample-major partitions
CP = 120         # class-major partitions (SLOT*C)
C = 10           # classes
SLOT = 12        # samples per column
NCH = 33         # chunks per tile (128 columns each)
COLS = NCH * P   # 4224 columns per tile
NT = 10          # tiles
SAMP_TILE = COLS * SLOT          # 50688 samples per tile
SHARD = NT * SAMP_TILE           # 506880 padded samples per core
NCORES = 8
PAD_VAL = 0.1    # pt == pe == 0.1 -> emd == 0 -> zero loss contribution

_CACHE = {}


def _build_nc():
    import concourse.bass as bass
    import concourse.tile as tile
    from concourse import bacc, mybir

    f32 = mybir.dt.float32
    f16 = mybir.dt.float16
    f8 = mybir.dt.float8e4
    Alu = mybir.AluOpType
    W = NT * COLS

    nc = bacc.Bacc("TRN2")
    pt_d = nc.dram_tensor("pt", [CP, W], f8, kind="ExternalInput").ap()
    pe_d = nc.dram_tensor("pe", [CP, W], f8, kind="ExternalInput").ap()
    tbd8_d = nc.dram_tensor("tbd8", [CP, CP], f8, kind="ExternalInput").ap()
    tbdn_d = nc.dram_tensor("tbdn", [CP, CP], f8, kind="ExternalInput").ap()
    ones_d = nc.dram_tensor("onesbd", [CP, SLOT], f16, kind="ExternalInput").ap()
    wst_d = nc.dram_tensor("wst", [CP, 2 * SLOT], f16, kind="ExternalInput").ap()
    out_d = nc.dram_tensor("out", [P, NT], f32, kind="ExternalOutput").ap()

    with tile.TileContext(nc) as tc:
        with (
            tc.tile_pool(name="consts", bufs=1) as cpool,
            tc.tile_pool(name="ins", bufs=6) as ipool,
            tc.tile_pool(name="dsq", bufs=12) as dpool,
            tc.tile_pool(name="fin", bufs=2) as spool,
            tc.tile_pool(name="psDa", bufs=3, space="PSUM") as ppDa,
            tc.tile_pool(name="psDp", bufs=1, space="PSUM") as ppDp,
            tc.tile_pool(name="psS", bufs=2, space="PSUM") as ppS,
            tc.tile_pool(name="psM", bufs=1, space="PSUM") as ppM,
            tc.tile_pool(name="outp", bufs=1) as opool,
        ):
            def load(t):
                ptt = ipool.tile([CP, COLS], f8, tag="ptt")
                nc.sync.dma_start(ptt[:], pt_d[:, t * COLS : (t + 1) * COLS])
                pet = ipool.tile([CP, COLS], f8, tag="pet")
                nc.sync.dma_start(pet[:], pe_d[:, t * COLS : (t + 1) * COLS])
                return ptt, pet

            preload = load(0)

            tbd8 = cpool.tile([CP, CP], f8, tag="tbd8")
            nc.sync.dma_start(tbd8[:], tbd8_d[:])
            tbdn = cpool.tile([CP, CP], f8, tag="tbdn")
            nc.sync.dma_start(tbdn[:], tbdn_d[:])
            onest = cpool.tile([CP, SLOT], f16, tag="onesbd")
            nc.sync.dma_start(onest[:], ones_d[:])
            wst = cpool.tile([CP, 2 * SLOT], f16, tag="wst")
            nc.sync.dma_start(wst[:], wst_d[:])

            acc = opool.tile([P, NT], f32, tag="acc")

            # chunks 0..17 -> psM_a, 18..32 -> psM_b (group-aligned so the
            # psMa copy can be issued mid-tile, right when chunk 17 drains)
            n_half = 18

            groups = [
                ("dve", 4), ("act", 4), ("pool", 3), ("act", 4),
                ("pool", 3), ("act", 4), ("pool", 3), ("act", 4),
                ("act", 4),
            ]
            # (order tuned empirically against the timeline model)
            starts = []
            c0 = 0
            for _, gch in groups:
                starts.append(c0)
                c0 += gch

            # cross-tile pipelining: tile t's ssq matmuls run during tile
            # t+1 (every square then has a full tile of slack), and the
            # emd/loss reduction for tile t completes early in tile t+1.
            prev = None

            def emit_ssq(prev):
                ppend, ppsS = prev["pend"], prev["psS"]
                for g2, (_, gch2) in enumerate(groups):
                    dsq2 = ppend[g2]
                    for j2 in range(gch2):
                        ch2 = starts[g2] + j2
                        nc.tensor.matmul(
                            ppsS[:, bass.ts(ch2, SLOT)],
                            dsq2[:, bass.ts(j2, P)],
                            onest[:],
                            start=True, stop=True,
                        )

            def emit_fin(prev):
                # full finishing chain for a completed tile: var = z2 - z1^2,
                # emd = sqrt(ssq/10), acc += emd / var
                momd, psS, t = prev["momd"], prev["psS"], prev["t"]
                z1 = momd.rearrange("p (k m) -> p k m", m=2)[:, :, 0]
                z2 = momd.rearrange("p (k m) -> p k m", m=2)[:, :, 1]
                zsq = spool.tile([P, NCH * SLOT], f32, tag="zsq")
                nc.gpsimd.tensor_tensor(zsq[:], z1, z1, op=Alu.mult)
                tv = spool.tile([P, NCH * SLOT], f32, tag="tv")
                nc.gpsimd.tensor_tensor(tv[:], z2, zsq[:], op=Alu.subtract)
                nc.vector.reciprocal_approx_fast(tv[:], tv[:])
                ssqm = spool.tile([P, NCH * SLOT], f32, tag="ssqm")
                nc.scalar.activation(
                    ssqm[:], psS[:],
                    mybir.ActivationFunctionType.Sqrt, scale=0.1,
                )
                scr = spool.tile([P, NCH * SLOT], f32, tag="scr")
                nc.vector.tensor_mul(scr[:], ssqm[:], tv[:])
                nc.vector.tensor_reduce(
                    acc[:, t : t + 1], scr[:],
                    axis=mybir.AxisListType.X, op=Alu.add,
                )

            for t in range(NT):
                ptt, pet = preload if t == 0 else load(t)

                psS = ppS.tile([P, NCH * SLOT], f32, tag="psS")
                psMa = ppM.tile([P, n_half * 2 * SLOT], f32, tag="psMa")
                psMb = ppM.tile([P, (NCH - n_half) * 2 * SLOT], f32, tag="psMb")

                # PE stream: an uninterrupted run of cdf matmul pairs with
                # the (square-independent) moment matmuls inlined; the ssq
                # matmuls of the PREVIOUS tile are interleaved one group at
                # a time -- their squares finished a full tile ago.
                #
                # The D^2 square is spread over the engines under the PSUM
                # access rules (GPSIMD: no PSUM; DVE: at most one PSUM input):
                #   act  -> ScalarE squares PSUM->SBUF directly
                #   pool -> DVE copies PSUM->SBUF fp16, GpSimd squares it
                #   dve  -> DVE copies PSUM->SBUF fp16, DVE squares it (2x)
                pend = []
                momd = spool.tile([P, NCH * 2 * SLOT], f16, tag="momd")

                for g, (eng, gch) in enumerate(groups):
                    gw = gch * P
                    ch0 = starts[g]
                    sl = slice(ch0 * P, ch0 * P + gw)
                    if eng == "pool":
                        psD = ppDp.tile([CP, 3 * P], f32, tag="psDp")
                    else:
                        psD = ppDa.tile([CP, 4 * P], f32, tag="psDa")
                    nc.tensor.matmul(
                        psD[:, :gw], tbd8[:], pet[:, sl], start=True, stop=False
                    )
                    nc.tensor.matmul(
                        psD[:, :gw], tbdn[:], ptt[:, sl], start=False, stop=True
                    )
                    for j in range(gch):
                        ch = ch0 + j
                        mdst = (
                            psMa[:, bass.ts(ch, 2 * SLOT)]
                            if ch < n_half
                            else psMb[:, bass.ts(ch - n_half, 2 * SLOT)]
                        )
                        nc.tensor.matmul(
                            mdst, ptt[:, bass.ts(ch, P)], wst[:],
                            start=True, stop=True,
                        )
                    if ch0 + gch == n_half:
                        nc.vector.tensor_copy(momd[:, : n_half * 2 * SLOT], psMa[:])
                    dsq = dpool.tile([CP, 4 * P], f16, tag="dsq")
                    if eng == "act":
                        nc.scalar.square(dsq[:, :gw], psD[:, :gw])
                    else:
                        dcp = dpool.tile([CP, 4 * P], f16, tag="dcp")
                        nc.vector.tensor_copy(dcp[:, :gw], psD[:, :gw])
                        if eng == "pool":
                            nc.gpsimd.tensor_mul(
                                dsq[:, :gw], dcp[:, :gw], dcp[:, :gw]
                            )
                        else:
                            nc.vector.tensor_mul(
                                dsq[:, :gw], dcp[:, :gw], dcp[:, :gw]
                            )
                    pend.append(dsq)
                    # previous tile's finishing chain slots in AFTER this
                    # tile's first square dispatch so Act's next psD-releasing
                    # square is not queued behind the previous sqrt
                    if g == 1 and prev is not None:
                        emit_fin(prev)

                emit_ssq({"pend": pend, "psS": psS})
                nc.scalar.copy(momd[:, n_half * 2 * SLOT :], psMb[:])
                prev = {"psS": psS, "momd": momd, "t": t}

            emit_fin(prev)
            nc.sync.dma_start(out_d[:], acc[:])

    nc.compile()
    return nc


def _consts():
    import ml_dtypes

    f8 = ml_dtypes.float8_e4m3
    # block-diagonal CDF transform: Tbd[(s,c),(s,k)] = 1 if c <= k
    tri = np.tril(np.ones((C, C), np.float32)).T  # [c, k]: 1 if c <= k
    tbd = np.zeros((CP, CP), np.float32)
    ones_bd = np.zeros((CP, SLOT), np.float16)
    wst = np.zeros((CP, 2 * SLOT), np.float16)
    wv1 = np.arange(1, C + 1, dtype=np.float64).astype(np.float16)
    wv2 = (np.arange(1, C + 1, dtype=np.float64) ** 2).astype(np.float16)
    for s in range(SLOT):
        tbd[s * C : (s + 1) * C, s * C : (s + 1) * C] = tri
        ones_bd[s * C : (s + 1) * C, s] = 1.0
        wst[s * C : (s + 1) * C, 2 * s] = wv1
        wst[s * C : (s + 1) * C, 2 * s + 1] = wv2
    return tbd.astype(f8), (-tbd).astype(f8), ones_bd, wst


def _shard(x, per, dtype):
    """[B, C] f32 -> per-core class-major [CP, NT*COLS] in dtype."""
    out = []
    for i in range(NCORES):
        s = np.asarray(x[i * per : (i + 1) * per])
        pad = SHARD - s.shape[0]
        if pad:
            s = np.concatenate([s, np.full((pad, C), PAD_VAL, s.dtype)], axis=0)
        # sample n = j*SLOT + s  ->  X[(s, c), j]
        v = s.reshape(NT * COLS, SLOT, C).transpose(1, 2, 0).reshape(CP, NT * COLS)
        out.append(np.ascontiguousarray(v).astype(dtype))
    return out


def kernel(p_target: np.ndarray, p_estimate: np.ndarray) -> np.ndarray:
    import ml_dtypes
    from concourse.bass_utils import run_bass_kernel_spmd

    if "nc" not in _CACHE:
        _CACHE["nc"] = _build_nc()
    nc = _CACHE["nc"]

    B = p_target.shape[0]
    per = B // NCORES
    tbd8, tbdn, ones_bd, wst = _consts()
    pt_sh = _shard(p_target, per, ml_dtypes.float8_e4m3)
    pe_sh = _shard(p_estimate, per, ml_dtypes.float8_e4m3)

    in_maps = [
        {
            "pt": pt_sh[i],
            "pe": pe_sh[i],
            "tbd8": tbd8,
            "tbdn": tbdn,
            "onesbd": ones_bd,
            "wst": wst,
        }
        for i in range(NCORES)
    ]
    res = run_bass_kernel_spmd(nc, in_maps, core_ids=list(range(NCORES)))
    total = sum(
        res.results[i]["out"].astype(np.float64).sum() for i in range(NCORES)
    )
    return np.float32(total / B)


# revision 50
# speedup vs baseline: 1.0065x; 1.0065x over previous
"""Balanced EMD loss kernel for Trainium2 (8 NeuronCores, data parallel).

Math (per sample, classes w = 1..10):
    score = sum(pt * w);  var = sum(pt * (w - score)^2) = Z2 - Z1^2  (S0 == 1)
    D_k = CDF_k(pe) - CDF_k(pt) = sum_{c<=k} (pe_c - pt_c)
    emd = sqrt(mean_k D_k^2);  loss = sum(emd / var) / B

Layout: class-major, host pre-transposed.  SBUF holds X[(s*10+c), j] =
x[sample(j*12+s), c] for 12 slots x 10 classes = 120 partitions; each
column j carries 12 samples.  pe is fp8 (e4m3), pt fp8 too -- the
loss is a mean over 4M samples so quantization noise averages out
(measured ~4e-3 against the f32 reference; the gate is 2e-2).

Engine split per tile (33 chunks of 128 columns, 9 square groups):
  TensorE: D = Tbd^T @ pe - Tbd^T @ pt via two accumulating matmuls with
           a constant block-diagonal lower-triangular stationary (the CDF
           transform); per chunk a tiny data-stationary moment matmul
           (z1,z2; moving = block-diag class weights) inlined with the cdf
           stream, and one ssq matmul (sum_k D^2 over the squared cdf;
           moving = block-diag ones) in a block at the tile end, by which
           time every square has finished.  Both land sample-major in PSUM.
  The D^2 square is spread over the other engines under the PSUM access
  rules (GPSIMD: no PSUM; DVE: at most one PSUM input per instruction):
  ScalarE squares 5 groups directly; DVE copies 4 groups to SBUF fp16, of
  which GpSimd squares 3 and DVE itself squares 1 (2x mode).
  Finishing per tile (sample-major [128, 396]), deferred into the next
  tile so it never blocks a psD-releasing square: GpSimd var = z2 - z1^2,
  VectorE 1/var + (emd*weight) + reduce, ScalarE emd = sqrt(0.1*ssq) and
  the moment-PSUM copies (split with VectorE).
"""

import numpy as np

P = 128          # sample-major partitions
CP = 120         # class-major partitions (SLOT*C)
C = 10           # classes
SLOT = 12        # samples per column
NCH = 33         # chunks per full tile (128 columns each)
COLS = NCH * P   # 4224 columns per full tile
NT = 10          # tiles; the last tile is short (29 chunks) to cut padding
TCH = [33] * 2 + [29] + [33] * 7  # chunks per tile
TOT_CH = sum(TCH)                # 326 chunks per core
W_COLS = TOT_CH * P              # 41728 columns per core
SHARD = W_COLS * SLOT            # 500736 padded samples per core
NCORES = 8
PAD_VAL = 0.1    # pt == pe == 0.1 -> emd == 0 -> zero loss contribution

_CACHE = {}


def _build_nc():
    import concourse.bass as bass
    import concourse.tile as tile
    from concourse import bacc, mybir

    f32 = mybir.dt.float32
    f16 = mybir.dt.float16
    f8 = mybir.dt.float8e4
    Alu = mybir.AluOpType
    W = W_COLS

    nc = bacc.Bacc("TRN2")
    pt_d = nc.dram_tensor("pt", [CP, W], f8, kind="ExternalInput").ap()
    pe_d = nc.dram_tensor("pe", [CP, W], f8, kind="ExternalInput").ap()
    tbd8_d = nc.dram_tensor("tbd8", [CP, CP], f8, kind="ExternalInput").ap()
    tbdn_d = nc.dram_tensor("tbdn", [CP, CP], f8, kind="ExternalInput").ap()
    ones_d = nc.dram_tensor("onesbd", [CP, SLOT], f16, kind="ExternalInput").ap()
    wst_d = nc.dram_tensor("wst", [CP, 2 * SLOT], f16, kind="ExternalInput").ap()
    out_d = nc.dram_tensor("out", [P, NT], f32, kind="ExternalOutput").ap()

    with tile.TileContext(nc) as tc:
        with (
            tc.tile_pool(name="consts", bufs=1) as cpool,
            tc.tile_pool(name="ins", bufs=6) as ipool,
            tc.tile_pool(name="dsq", bufs=12) as dpool,
            tc.tile_pool(name="fin", bufs=2) as spool,
            tc.tile_pool(name="psDa", bufs=3, space="PSUM") as ppDa,
            tc.tile_pool(name="psDp", bufs=1, space="PSUM") as ppDp,
            tc.tile_pool(name="psS", bufs=2, space="PSUM") as ppS,
            tc.tile_pool(name="psM", bufs=1, space="PSUM") as ppM,
            tc.tile_pool(name="outp", bufs=1) as opool,
        ):
            toff = [0]
            for tch in TCH:
                toff.append(toff[-1] + tch)

            def load(t):
                lo, w = toff[t] * P, TCH[t] * P
                ptt = ipool.tile([CP, COLS], f8, tag="ptt")
                nc.sync.dma_start(ptt[:, :w], pt_d[:, lo : lo + w])
                pet = ipool.tile([CP, COLS], f8, tag="pet")
                nc.sync.dma_start(pet[:, :w], pe_d[:, lo : lo + w])
                return ptt, pet

            preload = load(0)

            tbd8 = cpool.tile([CP, CP], f8, tag="tbd8")
            nc.sync.dma_start(tbd8[:], tbd8_d[:])
            tbdn = cpool.tile([CP, CP], f8, tag="tbdn")
            nc.sync.dma_start(tbdn[:], tbdn_d[:])
            onest = cpool.tile([CP, SLOT], f16, tag="onesbd")
            nc.sync.dma_start(onest[:], ones_d[:])
            wst = cpool.tile([CP, 2 * SLOT], f16, tag="wst")
            nc.sync.dma_start(wst[:], wst_d[:])

            acc = opool.tile([P, NT], f32, tag="acc")

            # chunks 0..17 -> psM_a, 18..32 -> psM_b (group-aligned so the
            # psMa copy can be issued mid-tile, right when chunk 17 drains)
            n_half = 18

            groups_full = [
                ("dve", 4), ("act", 4), ("pool", 3), ("act", 4),
                ("pool", 3), ("act", 4), ("pool", 3), ("act", 4),
                ("act", 4),
            ]
            # short tile drops the trailing 4-chunk act group
            groups_short = groups_full[:-1]
            # (order tuned empirically against the timeline model)

            def mk_starts(groups):
                starts, c0 = [], 0
                for _, gch in groups:
                    starts.append(c0)
                    c0 += gch
                return starts

            # cross-tile pipelining: tile t's ssq matmuls run during tile
            # t+1 (every square then has a full tile of slack), and the
            # emd/loss reduction for tile t completes early in tile t+1.
            prev = None

            def emit_ssq(prev):
                ppend, ppsS = prev["pend"], prev["psS"]
                pgroups, pstarts = prev["groups"], prev["starts"]
                for g2, (_, gch2) in enumerate(pgroups):
                    dsq2 = ppend[g2]
                    for j2 in range(gch2):
                        ch2 = pstarts[g2] + j2
                        nc.tensor.matmul(
                            ppsS[:, bass.ts(ch2, SLOT)],
                            dsq2[:, bass.ts(j2, P)],
                            onest[:],
                            start=True, stop=True,
                        )

            def emit_fin(prev):
                # full finishing chain for a completed tile: var = z2 - z1^2,
                # emd = sqrt(ssq/10), acc += emd / var
                momd, psS, t = prev["momd"], prev["psS"], prev["t"]
                n = prev["tch"] * SLOT
                z1 = momd[:, : 2 * n].rearrange("p (k m) -> p k m", m=2)[:, :, 0]
                z2 = momd[:, : 2 * n].rearrange("p (k m) -> p k m", m=2)[:, :, 1]
                zsq = spool.tile([P, NCH * SLOT], f32, tag="zsq")
                nc.gpsimd.tensor_tensor(zsq[:, :n], z1, z1, op=Alu.mult)
                tv = spool.tile([P, NCH * SLOT], f32, tag="tv")
                nc.gpsimd.tensor_tensor(tv[:, :n], z2, zsq[:, :n], op=Alu.subtract)
                nc.vector.reciprocal_approx_fast(tv[:, :n], tv[:, :n])
                ssqm = spool.tile([P, NCH * SLOT], f32, tag="ssqm")
                nc.scalar.activation(
                    ssqm[:, :n], psS[:, :n],
                    mybir.ActivationFunctionType.Sqrt, scale=0.1,
                )
                scr = spool.tile([P, NCH * SLOT], f32, tag="scr")
                nc.vector.tensor_mul(scr[:, :n], ssqm[:, :n], tv[:, :n])
                nc.vector.tensor_reduce(
                    acc[:, t : t + 1], scr[:, :n],
                    axis=mybir.AxisListType.X, op=Alu.add,
                )

            for t in range(NT):
                ptt, pet = preload if t == 0 else load(t)
                groups = groups_full if TCH[t] == NCH else groups_short
                starts = mk_starts(groups)

                psS = ppS.tile([P, NCH * SLOT], f32, tag="psS")
                psMa = ppM.tile([P, n_half * 2 * SLOT], f32, tag="psMa")
                psMb = ppM.tile([P, (NCH - n_half) * 2 * SLOT], f32, tag="psMb")

                # PE stream: an uninterrupted run of cdf matmul pairs with
                # the (square-independent) moment matmuls inlined; the ssq
                # matmuls of the PREVIOUS tile are interleaved one group at
                # a time -- their squares finished a full tile ago.
                #
                # The D^2 square is spread over the engines under the PSUM
                # access rules (GPSIMD: no PSUM; DVE: at most one PSUM input):
                #   act  -> ScalarE squares PSUM->SBUF directly
                #   pool -> DVE copies PSUM->SBUF fp16, GpSimd squares it
                #   dve  -> DVE copies PSUM->SBUF fp16, DVE squares it (2x)
                pend = []
                momd = spool.tile([P, NCH * 2 * SLOT], f16, tag="momd")

                for g, (eng, gch) in enumerate(groups):
                    gw = gch * P
                    ch0 = starts[g]
                    sl = slice(ch0 * P, ch0 * P + gw)
                    if eng == "pool":
                        psD = ppDp.tile([CP, 3 * P], f32, tag="psDp")
                    else:
                        psD = ppDa.tile([CP, 4 * P], f32, tag="psDa")
                    nc.tensor.matmul(
                        psD[:, :gw], tbd8[:], pet[:, sl], start=True, stop=False
                    )
                    nc.tensor.matmul(
                        psD[:, :gw], tbdn[:], ptt[:, sl], start=False, stop=True
                    )
                    for j in range(gch):
                        ch = ch0 + j
                        mdst = (
                            psMa[:, bass.ts(ch, 2 * SLOT)]
                            if ch < n_half
                            else psMb[:, bass.ts(ch - n_half, 2 * SLOT)]
                        )
                        nc.tensor.matmul(
                            mdst, ptt[:, bass.ts(ch, P)], wst[:],
                            start=True, stop=True,
                        )
                    if ch0 + gch == n_half:
                        nc.vector.tensor_copy(momd[:, : n_half * 2 * SLOT], psMa[:])
                    dsq = dpool.tile([CP, 4 * P], f16, tag="dsq")
                    if eng == "act":
                        nc.scalar.square(dsq[:, :gw], psD[:, :gw])
                    else:
                        dcp = dpool.tile([CP, 4 * P], f16, tag="dcp")
                        nc.vector.tensor_copy(dcp[:, :gw], psD[:, :gw])
                        if eng == "pool":
                            nc.gpsimd.tensor_mul(
                                dsq[:, :gw], dcp[:, :gw], dcp[:, :gw]
                            )
                        else:
                            nc.vector.tensor_mul(
                                dsq[:, :gw], dcp[:, :gw], dcp[:, :gw]
                            )
                    pend.append(dsq)
                    # previous tile's finishing chain slots in AFTER this
                    # tile's first square dispatch so Act's next psD-releasing
                    # square is not queued behind the previous sqrt
                    if g == 1 and prev is not None:
                        emit_fin(prev)

                emit_ssq(
                    {"pend": pend, "psS": psS, "groups": groups, "starts": starts}
                )
                nc.scalar.copy(
                    momd[:, n_half * 2 * SLOT : TCH[t] * 2 * SLOT],
                    psMb[:, : (TCH[t] - n_half) * 2 * SLOT],
                )
                prev = {"psS": psS, "momd": momd, "t": t, "tch": TCH[t]}

            emit_fin(prev)
            nc.sync.dma_start(out_d[:], acc[:])

    nc.compile()
    return nc


def _consts():
    import ml_dtypes

    f8 = ml_dtypes.float8_e4m3
    # block-diagonal CDF transform: Tbd[(s,c),(s,k)] = 1 if c <= k
    tri = np.tril(np.ones((C, C), np.float32)).T  # [c, k]: 1 if c <= k
    tbd = np.zeros((CP, CP), np.float32)
    ones_bd = np.zeros((CP, SLOT), np.float16)
    wst = np.zeros((CP, 2 * SLOT), np.float16)
    wv1 = np.arange(1, C + 1, dtype=np.float64).astype(np.float16)
    wv2 = (np.arange(1, C + 1, dtype=np.float64) ** 2).astype(np.float16)
    for s in range(SLOT):
        tbd[s * C : (s + 1) * C, s * C : (s + 1) * C] = tri
        ones_bd[s * C : (s + 1) * C, s] = 1.0
        wst[s * C : (s + 1) * C, 2 * s] = wv1
        wst[s * C : (s + 1) * C, 2 * s + 1] = wv2
    return tbd.astype(f8), (-tbd).astype(f8), ones_bd, wst


def _shard(x, per, dtype):
    """[B, C] f32 -> per-core class-major [CP, NT*COLS] in dtype."""
    out = []
    for i in range(NCORES):
        s = np.asarray(x[i * per : (i + 1) * per])
        pad = SHARD - s.shape[0]
        if pad:
            s = np.concatenate([s, np.full((pad, C), PAD_VAL, s.dtype)], axis=0)
        # sample n = j*SLOT + s  ->  X[(s, c), j]
        v = s.reshape(W_COLS, SLOT, C).transpose(1, 2, 0).reshape(CP, W_COLS)
        out.append(np.ascontiguousarray(v).astype(dtype))
    return out


def kernel(p_target: np.ndarray, p_estimate: np.ndarray) -> np.ndarray:
    import ml_dtypes
    from concourse.bass_utils import run_bass_kernel_spmd

    if "nc" not in _CACHE:
        _CACHE["nc"] = _build_nc()
    nc = _CACHE["nc"]

    B = p_target.shape[0]
    per = B // NCORES
    tbd8, tbdn, ones_bd, wst = _consts()
    pt_sh = _shard(p_target, per, ml_dtypes.float8_e4m3)
    pe_sh = _shard(p_estimate, per, ml_dtypes.float8_e4m3)

    in_maps = [
        {
            "pt": pt_sh[i],
            "pe": pe_sh[i],
            "tbd8": tbd8,
            "tbdn": tbdn,
            "onesbd": ones_bd,
            "wst": wst,
        }
        for i in range(NCORES)
    ]
    res = run_bass_kernel_spmd(nc, in_maps, core_ids=list(range(NCORES)))
    total = sum(
        res.results[i]["out"].astype(np.float64).sum() for i in range(NCORES)
    )
    return np.float32(total / B)
